# revision 20
# baseline (speedup 1.0000x reference)
"""BreadthAttentionConv (GNN attention message passing) on 8 Trainium2 cores.

Sharding: destination-node partition. Core c owns N/8 consecutive dst nodes
and processes exactly the edges pointing into them, so the segment softmax and
the weighted scatter-sum are core-local (no collectives).

Host-side layout: per core, nodes are sorted by in-degree and grouped into
blocks of 128 (the SBUF partition dim). Each node's incoming-edge list is
padded to the block's slot count D_b (schedule shared by all cores so the
SPMD program is identical). The host ships h[src] for every (node, slot) in
slot-column-major order, so the device needs no gather at all.

Device, per block b and slot-column g (128 nodes x D_b slots):
  lhsT[:, g] = [h_src(node,g); h_dst(node)]          (128-dim stacked input)
  psum = lhsT.T @ [[Wd.T;Ws.T] | [Wmsg.T;0]]        -> [z | hm] per slot
  t = tanh(z); e = t . v; p = exp(e + mask)
  out = tanh((sum_g p*hm) / (sum_g p))
"""
import sys

for _p in ("/opt/trn_rl_repo",):
    if _p not in sys.path:
        sys.path.insert(0, _p)

import numpy as np

import concourse.bass as bass
import concourse.bacc as bacc
import concourse.tile as tile
from concourse import mybir
from concourse.bass_utils import run_bass_kernel_spmd

P = 128
NCORES = 8
MASK_NEG = -30.0
SUBG = 16  # slot-columns per PSUM sub-batch


# ---------------------------------------------------------------- host side
def _make_plan(deg_sorted_by_core):
    heads = deg_sorted_by_core[:, ::P]
    d = heads.max(axis=0)
    d = np.maximum(d, 1)
    d = ((d + 1) // 2) * 2
    return d.astype(np.int64)


def _preprocess(h, edge_index, W_msg, Ws, Wd, v, ncores):
    n, in_dim = h.shape
    own = n // ncores
    n_blocks = (own + P - 1) // P
    own_pad = n_blocks * P

    ei = np.asarray(edge_index)
    loops = np.arange(n, dtype=ei.dtype)
    src = np.concatenate([ei[0], loops]).astype(np.int64)
    dst = np.concatenate([ei[1], loops]).astype(np.int64)

    deg = np.bincount(dst, minlength=n)
    core_of = dst // own

    perms = []
    deg_sorted = np.zeros((ncores, own_pad), dtype=np.int64)
    for c in range(ncores):
        d_c = deg[c * own : (c + 1) * own]
        perm = np.argsort(-d_c, kind="stable")
        perms.append(perm)
        deg_sorted[c, :own] = d_c[perm]
    d_blocks = _make_plan(deg_sorted)
    col_of_block = np.zeros(n_blocks + 1, dtype=np.int64)
    np.cumsum(d_blocks, out=col_of_block[1:])
    s_total = int(col_of_block[-1])

    h32 = np.asarray(h, dtype=np.float32)
    h16 = h32.astype(np.float16)
    # stacked weights: [ [Wd.T ; Ws.T] | [Wmsg.T ; 0] ]  -> [2*in, a+out]
    wz = np.concatenate([np.asarray(Wd).T, np.asarray(Ws).T], axis=0)
    wm = np.concatenate(
        [np.asarray(W_msg).T, np.zeros_like(np.asarray(W_msg).T)], axis=0
    )
    wsd = np.ascontiguousarray(
        np.concatenate([wz, wm], axis=1).astype(np.float16)
    )  # [128, 128]
    vb = np.ascontiguousarray(
        np.tile(np.asarray(v).astype(np.float16), (P, 1))
    )

    in_maps = []
    for c in range(ncores):
        m = core_of == c
        src_c = src[m]
        dst_local = dst[m] - c * own
        perm = perms[c]
        rank = np.empty(own, dtype=np.int64)
        rank[perm] = np.arange(own)
        key = rank[dst_local]
        order = np.argsort(key, kind="stable")
        src_sorted = src_c[order]
        key_sorted = key[order]
        counts = np.bincount(key_sorted, minlength=own_pad)
        starts = np.zeros(own_pad + 1, dtype=np.int64)
        np.cumsum(counts, out=starts[1:])
        slot = np.arange(len(key_sorted)) - starts[key_sorted]
        blk = key_sorted // P
        part = key_sorted % P
        col = col_of_block[blk] * P + slot * P + part  # slot-column-major pos

        src_of_pos = np.zeros(s_total * P, dtype=np.int64)  # pad -> node 0
        src_of_pos[col] = src_sorted
        mask = np.full((P, s_total), MASK_NEG, dtype=np.float32)
        mask[part, col_of_block[blk] + slot] = 0.0
        for r in range(own, own_pad):
            mask[r % P, col_of_block[r // P]] = 0.0

        # h_srcT: [in_dim, s_total*128] fp16, column q holds h[src_of_pos[q]]
        h_srcT = np.ascontiguousarray(h16[src_of_pos].T)
        hp = np.zeros((own_pad, in_dim), dtype=np.float16)
        hp[:own] = h16[c * own : (c + 1) * own][perm]
        hpT = np.ascontiguousarray(hp.T)
        in_maps.append(
            {
                "hsrcT": h_srcT,
                "hpT": hpT,
                "wsd": wsd,
                "vb": vb,
                "mask": mask,
            }
        )
    meta = dict(
        n=n, own=own, own_pad=own_pad, n_blocks=n_blocks,
        d_blocks=d_blocks, col_of_block=col_of_block, perms=perms,
    )
    return in_maps, meta


# ---------------------------------------------------------------- device side
def _build_program(n_blocks, d_blocks, col_of_block, own_pad, in_dim=64,
                   a_dim=64, out_dim=64):
    f16, f32 = mybir.dt.float16, mybir.dt.float32
    in2 = 2 * in_dim  # stacked input dim (128)
    odim2 = a_dim + out_dim  # psum row width (128)
    s_total = int(col_of_block[-1])

    nc = bacc.Bacc("TRN2", target_bir_lowering=False, debug=False)
    hsrcT = nc.dram_tensor(
        "hsrcT", [in_dim, s_total * P], f16, kind="ExternalInput"
    )
    hpT_d = nc.dram_tensor("hpT", [in_dim, own_pad], f16, kind="ExternalInput")
    wsd_d = nc.dram_tensor("wsd", [in2, odim2], f16, kind="ExternalInput")
    vb_d = nc.dram_tensor("vb", [P, a_dim], f16, kind="ExternalInput")
    mask_d = nc.dram_tensor("mask", [P, s_total], f32, kind="ExternalInput")
    out_d = nc.dram_tensor(
        "out", [own_pad, out_dim], f32, kind="ExternalOutput"
    )

    with tile.TileContext(nc) as tc:
        with (
            tc.tile_pool(name="consts", bufs=1) as consts,
            tc.tile_pool(name="lhs", bufs=3) as lhs,
            tc.tile_pool(name="psum", bufs=2, space="PSUM") as psum,
            tc.tile_pool(name="work", bufs=3) as work,
            tc.tile_pool(name="small", bufs=3) as small,
            tc.tile_pool(name="outp", bufs=3) as outp,
        ):
            wsd_sb = consts.tile([in2, odim2], f16)
            nc.sync.dma_start(out=wsd_sb[:], in_=wsd_d[:])
            vb_sb = consts.tile([P, a_dim], f16)
            nc.sync.dma_start(out=vb_sb[:], in_=vb_d[:])
            mask_sb = consts.tile([P, s_total], f32)
            nc.sync.dma_start(out=mask_sb[:], in_=mask_d[:])

            ob_group = 8
            out_t = None
            for b in range(n_blocks):
                db = int(d_blocks[b])
                off = int(col_of_block[b])
                ts = lhs.tile([in2, db * P], f16, tag="ts")
                # top half: streamed h_src slot-columns
                nc.sync.dma_start(
                    out=ts[:in_dim, :],
                    in_=hsrcT[:, off * P : (off + db) * P],
                )
                # bottom half: h_dst block replicated across slot-columns
                nc.sync.dma_start(
                    out=ts[in_dim:, :].rearrange("p (g n) -> p g n", n=P),
                    in_=bass.AP(
                        tensor=hpT_d,
                        offset=b * P,
                        ap=[[own_pad, in_dim], [0, db], [1, P]],
                    ),
                )
                # per-block working tiles
                t_sb = work.tile([P, db * a_dim], f16, tag="t")
                hm_sb = work.tile([P, db * out_dim], f16, tag="hm")
                w_sb = work.tile([P, db * out_dim], f16, tag="w")
                e_sb = small.tile([P, db], f32, tag="e")
                p_sb = small.tile([P, db], f16, tag="p")
                n_sub = (db + SUBG - 1) // SUBG
                for sb_i in range(n_sub):
                    g0 = sb_i * SUBG
                    gn = min(SUBG, db - g0)
                    pz = psum.tile([P, SUBG * odim2], f32, tag="pz")
                    for g in range(gn):
                        nc.tensor.matmul(
                            out=pz[:, g * odim2 : (g + 1) * odim2],
                            lhsT=ts[:, (g0 + g) * P : (g0 + g + 1) * P],
                            rhs=wsd_sb[:],
                            start=True,
                            stop=True,
                        )
                    pzv = pz[:].rearrange("p (g d) -> p g d", d=odim2)
                    t_v = t_sb[:].rearrange("p (g d) -> p g d", d=a_dim)
                    hm_v = hm_sb[:].rearrange("p (g d) -> p g d", d=out_dim)
                    w_v = w_sb[:].rearrange("p (g d) -> p g d", d=out_dim)
                    # tanh(z) for gn columns -> t_sb (ACT, psum read)
                    nc.scalar.activation(
                        out=t_v[:, g0 : g0 + gn, :],
                        in_=pzv[:, :gn, :a_dim],
                        func=mybir.ActivationFunctionType.Tanh,
                    )
                    # evict hm half of psum -> contiguous fp16 (ACT copy)
                    nc.scalar.activation(
                        out=hm_v[:, g0 : g0 + gn, :],
                        in_=pzv[:, :gn, a_dim:],
                        func=mybir.ActivationFunctionType.Copy,
                    )
                    tv = work.tile([P, SUBG * a_dim], f16, tag="tv")
                    nc.vector.tensor_tensor(
                        out=tv[:].rearrange("p (g d) -> p g d", d=a_dim)[
                            :, :gn, :
                        ],
                        in0=t_v[:, g0 : g0 + gn, :],
                        in1=vb_sb[:].unsqueeze(1).to_broadcast(
                            [P, gn, a_dim]
                        ),
                        op=mybir.AluOpType.mult,
                    )
                    nc.vector.tensor_reduce(
                        out=e_sb[:, g0 : g0 + gn],
                        in_=tv[:].rearrange("p (g d) -> p g d", d=a_dim)[
                            :, :gn, :
                        ],
                        axis=mybir.AxisListType.X,
                        op=mybir.AluOpType.add,
                    )
                    e2 = small.tile([P, SUBG], f32, tag="e2")
                    nc.vector.tensor_tensor(
                        out=e2[:, :gn],
                        in0=e_sb[:, g0 : g0 + gn],
                        in1=mask_sb[:, off + g0 : off + g0 + gn],
                        op=mybir.AluOpType.add,
                    )
                    nc.scalar.activation(
                        out=p_sb[:, g0 : g0 + gn],
                        in_=e2[:, :gn],
                        func=mybir.ActivationFunctionType.Exp,
                    )
                    # w[p, (g,d)] = p[p, g] * hm[p, (g,d)]
                    nc.vector.tensor_tensor(
                        out=w_v[:, g0 : g0 + gn, :],
                        in0=hm_v[:, g0 : g0 + gn, :],
                        in1=p_sb[:, g0 : g0 + gn]
                        .unsqueeze(2)
                        .to_broadcast([P, gn, out_dim]),
                        op=mybir.AluOpType.mult,
                    )
                    # accumulate later sub-batch slabs onto the first (one
                    # wide contiguous fp16 add each; 2x mode)
                    if sb_i > 0:
                        nc.vector.tensor_tensor(
                            out=w_sb[:, : gn * out_dim],
                            in0=w_sb[:, : gn * out_dim],
                            in1=w_sb[:, g0 * out_dim : (g0 + gn) * out_dim],
                            op=mybir.AluOpType.add,
                        )
                # fold-tree sum over the first slab, once per block:
                # contiguous fp16 halving adds stay in the DVE 2x mode
                gf = min(SUBG, db)
                while gf > 1:
                    if gf % 2 == 1:
                        nc.vector.tensor_tensor(
                            out=w_sb[:, :out_dim],
                            in0=w_sb[:, :out_dim],
                            in1=w_sb[:, (gf - 1) * out_dim : gf * out_dim],
                            op=mybir.AluOpType.add,
                        )
                        gf -= 1
                    half = gf // 2
                    nc.vector.tensor_tensor(
                        out=w_sb[:, : half * out_dim],
                        in0=w_sb[:, : half * out_dim],
                        in1=w_sb[:, half * out_dim : 2 * half * out_dim],
                        op=mybir.AluOpType.add,
                    )
                    gf = half
                denom = small.tile([P, 1], f32, tag="denom")
                nc.vector.tensor_reduce(
                    out=denom[:], in_=p_sb[:], axis=mybir.AxisListType.X,
                    op=mybir.AluOpType.add,
                )
                r_sb = small.tile([P, 1], f32, tag="r")
                nc.vector.reciprocal(out=r_sb[:], in_=denom[:])
                gi = b % ob_group
                if gi == 0:
                    out_t = outp.tile([P, ob_group * out_dim], f32, tag="ot")
                # out = tanh(numer * (1/denom)): the scale rides on ACT
                nc.scalar.activation(
                    out=out_t[:, gi * out_dim : (gi + 1) * out_dim],
                    in_=w_sb[:, :out_dim],
                    func=mybir.ActivationFunctionType.Tanh,
                    scale=r_sb[:],
                )
                if gi == ob_group - 1 or b == n_blocks - 1:
                    ng = gi + 1
                    b0 = b - gi
                    nc.sync.dma_start(
                        out=bass.AP(
                            tensor=out_d,
                            offset=b0 * P * out_dim,
                            ap=[[out_dim, P], [P * out_dim, ng], [1, out_dim]],
                        ),
                        in_=out_t[:].rearrange("p (g d) -> p g d", d=out_dim)[
                            :, :ng, :
                        ],
                    )
    nc.compile()
    return nc


_CACHE = {}


def _get_program(meta):
    key = (
        meta["own_pad"], meta["n_blocks"],
        tuple(int(x) for x in meta["d_blocks"]),
    )
    if key not in _CACHE:
        _CACHE[key] = _build_program(
            meta["n_blocks"], meta["d_blocks"], meta["col_of_block"],
            meta["own_pad"],
        )
    return _CACHE[key]


def run(h, edge_index, W_msg, Ws, Wd, v, trace=False, trace_kwargs=None):
    in_maps, meta = _preprocess(h, edge_index, W_msg, Ws, Wd, v, NCORES)
    nc = _get_program(meta)
    kwargs = {}
    if trace:
        kwargs = dict(trace=True, **(trace_kwargs or {}))
    res = run_bass_kernel_spmd(nc, in_maps, list(range(NCORES)), **kwargs)
    n, own = meta["n"], meta["own"]
    out_dim = res.results[0]["out"].shape[1]
    full = np.zeros((n, out_dim), dtype=np.float32)
    for c in range(NCORES):
        perm = meta["perms"][c]
        full[c * own + perm] = res.results[c]["out"][:own]
    return full, res


def kernel(h, edge_index, W_msg, Ws, Wd, v):
    out, _ = run(h, edge_index, W_msg, Ws, Wd, v)
    return out


# revision 21
# speedup vs baseline: 1.0023x; 1.0023x over previous
"""BreadthAttentionConv (GNN attention message passing) on 8 Trainium2 cores.

Sharding: destination-node partition. Core c owns N/8 consecutive dst nodes
and processes exactly the edges pointing into them, so the segment softmax and
the weighted scatter-sum are core-local (no collectives).

Host-side layout: per core, nodes are sorted by in-degree and grouped into
blocks of 128 (the SBUF partition dim). Each node's incoming-edge list is
padded to the block's slot count D_b (schedule shared by all cores so the
SPMD program is identical). The host ships h[src] for every (node, slot) in
slot-column-major order, so the device needs no gather at all.

Device, per block b and slot-column g (128 nodes x D_b slots):
  lhsT[:, g] = [h_src(node,g); h_dst(node)]          (128-dim stacked input)
  psum = lhsT.T @ [[Wd.T;Ws.T] | [Wmsg.T;0]]        -> [z | hm] per slot
  t = tanh(z); e = t . v; p = exp(e + mask)
  out = tanh((sum_g p*hm) / (sum_g p))
"""
import sys

for _p in ("/opt/trn_rl_repo",):
    if _p not in sys.path:
        sys.path.insert(0, _p)

import numpy as np

import concourse.bass as bass
import concourse.bacc as bacc
import concourse.tile as tile
from concourse import mybir
from concourse.bass_utils import run_bass_kernel_spmd

P = 128
NCORES = 8
MASK_NEG = -30.0
SUBG = 16  # slot-columns per PSUM sub-batch


# ---------------------------------------------------------------- host side
def _make_plan(deg_sorted_by_core):
    heads = deg_sorted_by_core[:, ::P]
    d = heads.max(axis=0)
    d = np.maximum(d, 1)
    d = ((d + 1) // 2) * 2
    return d.astype(np.int64)


def _preprocess(h, edge_index, W_msg, Ws, Wd, v, ncores):
    n, in_dim = h.shape
    own = n // ncores
    n_blocks = (own + P - 1) // P
    own_pad = n_blocks * P

    ei = np.asarray(edge_index)
    loops = np.arange(n, dtype=ei.dtype)
    src = np.concatenate([ei[0], loops]).astype(np.int64)
    dst = np.concatenate([ei[1], loops]).astype(np.int64)

    deg = np.bincount(dst, minlength=n)
    core_of = dst // own

    perms = []
    deg_sorted = np.zeros((ncores, own_pad), dtype=np.int64)
    for c in range(ncores):
        d_c = deg[c * own : (c + 1) * own]
        perm = np.argsort(-d_c, kind="stable")
        perms.append(perm)
        deg_sorted[c, :own] = d_c[perm]
    d_blocks = _make_plan(deg_sorted)
    col_of_block = np.zeros(n_blocks + 1, dtype=np.int64)
    np.cumsum(d_blocks, out=col_of_block[1:])
    s_total = int(col_of_block[-1])

    h32 = np.asarray(h, dtype=np.float32)
    h16 = h32.astype(np.float16)
    # stacked weights: [ [Wd.T ; Ws.T] | [Wmsg.T ; 0] ]  -> [2*in, a+out]
    wz = np.concatenate([np.asarray(Wd).T, np.asarray(Ws).T], axis=0)
    wm = np.concatenate(
        [np.asarray(W_msg).T, np.zeros_like(np.asarray(W_msg).T)], axis=0
    )
    wsd = np.ascontiguousarray(
        np.concatenate([wz, wm], axis=1).astype(np.float16)
    )  # [128, 128]
    vb = np.ascontiguousarray(
        np.tile(np.asarray(v).astype(np.float16), (P, 1))
    )

    in_maps = []
    for c in range(ncores):
        m = core_of == c
        src_c = src[m]
        dst_local = dst[m] - c * own
        perm = perms[c]
        rank = np.empty(own, dtype=np.int64)
        rank[perm] = np.arange(own)
        key = rank[dst_local]
        order = np.argsort(key, kind="stable")
        src_sorted = src_c[order]
        key_sorted = key[order]
        counts = np.bincount(key_sorted, minlength=own_pad)
        starts = np.zeros(own_pad + 1, dtype=np.int64)
        np.cumsum(counts, out=starts[1:])
        slot = np.arange(len(key_sorted)) - starts[key_sorted]
        blk = key_sorted // P
        part = key_sorted % P
        col = col_of_block[blk] * P + slot * P + part  # slot-column-major pos

        src_of_pos = np.zeros(s_total * P, dtype=np.int64)  # pad -> node 0
        src_of_pos[col] = src_sorted
        mask = np.full((P, s_total), MASK_NEG, dtype=np.float32)
        mask[part, col_of_block[blk] + slot] = 0.0
        for r in range(own, own_pad):
            mask[r % P, col_of_block[r // P]] = 0.0

        # h_srcT: [in_dim, s_total*128] fp16, column q holds h[src_of_pos[q]]
        h_srcT = np.ascontiguousarray(h16[src_of_pos].T)
        hp = np.zeros((own_pad, in_dim), dtype=np.float16)
        hp[:own] = h16[c * own : (c + 1) * own][perm]
        hpT = np.ascontiguousarray(hp.T)
        in_maps.append(
            {
                "hsrcT": h_srcT,
                "hpT": hpT,
                "wsd": wsd,
                "vb": vb,
                "mask": mask,
            }
        )
    meta = dict(
        n=n, own=own, own_pad=own_pad, n_blocks=n_blocks,
        d_blocks=d_blocks, col_of_block=col_of_block, perms=perms,
    )
    return in_maps, meta


# ---------------------------------------------------------------- device side
def _build_program(n_blocks, d_blocks, col_of_block, own_pad, in_dim=64,
                   a_dim=64, out_dim=64):
    f16, f32 = mybir.dt.float16, mybir.dt.float32
    in2 = 2 * in_dim  # stacked input dim (128)
    odim2 = a_dim + out_dim  # psum row width (128)
    s_total = int(col_of_block[-1])

    nc = bacc.Bacc("TRN2", target_bir_lowering=False, debug=False)
    hsrcT = nc.dram_tensor(
        "hsrcT", [in_dim, s_total * P], f16, kind="ExternalInput"
    )
    hpT_d = nc.dram_tensor("hpT", [in_dim, own_pad], f16, kind="ExternalInput")
    wsd_d = nc.dram_tensor("wsd", [in2, odim2], f16, kind="ExternalInput")
    vb_d = nc.dram_tensor("vb", [P, a_dim], f16, kind="ExternalInput")
    mask_d = nc.dram_tensor("mask", [P, s_total], f32, kind="ExternalInput")
    out_d = nc.dram_tensor(
        "out", [own_pad, out_dim], f32, kind="ExternalOutput"
    )

    with tile.TileContext(nc) as tc:
        with (
            tc.tile_pool(name="consts", bufs=1) as consts,
            tc.tile_pool(name="lhs", bufs=3) as lhs,
            tc.tile_pool(name="psum", bufs=2, space="PSUM") as psum,
            tc.tile_pool(name="work", bufs=4) as work,
            tc.tile_pool(name="small", bufs=4) as small,
            tc.tile_pool(name="outp", bufs=4) as outp,
        ):
            wsd_sb = consts.tile([in2, odim2], f16)
            nc.sync.dma_start(out=wsd_sb[:], in_=wsd_d[:])
            vb_sb = consts.tile([P, a_dim], f16)
            nc.sync.dma_start(out=vb_sb[:], in_=vb_d[:])
            mask_sb = consts.tile([P, s_total], f32)
            nc.sync.dma_start(out=mask_sb[:], in_=mask_d[:])

            ob_group = 8
            out_t = None
            for b in range(n_blocks):
                db = int(d_blocks[b])
                off = int(col_of_block[b])
                ts = lhs.tile([in2, db * P], f16, tag="ts")
                # top half: streamed h_src slot-columns
                nc.sync.dma_start(
                    out=ts[:in_dim, :],
                    in_=hsrcT[:, off * P : (off + db) * P],
                )
                # bottom half: h_dst block replicated across slot-columns
                nc.sync.dma_start(
                    out=ts[in_dim:, :].rearrange("p (g n) -> p g n", n=P),
                    in_=bass.AP(
                        tensor=hpT_d,
                        offset=b * P,
                        ap=[[own_pad, in_dim], [0, db], [1, P]],
                    ),
                )
                # per-block working tiles
                t_sb = work.tile([P, db * a_dim], f16, tag="t")
                hm_sb = work.tile([P, db * out_dim], f16, tag="hm")
                w_sb = work.tile([P, db * out_dim], f16, tag="w")
                e_sb = small.tile([P, db], f32, tag="e")
                p_sb = small.tile([P, db], f16, tag="p")
                n_sub = (db + SUBG - 1) // SUBG
                for sb_i in range(n_sub):
                    g0 = sb_i * SUBG
                    gn = min(SUBG, db - g0)
                    pz = psum.tile([P, SUBG * odim2], f32, tag="pz")
                    for g in range(gn):
                        nc.tensor.matmul(
                            out=pz[:, g * odim2 : (g + 1) * odim2],
                            lhsT=ts[:, (g0 + g) * P : (g0 + g + 1) * P],
                            rhs=wsd_sb[:],
                            start=True,
                            stop=True,
                        )
                    pzv = pz[:].rearrange("p (g d) -> p g d", d=odim2)
                    t_v = t_sb[:].rearrange("p (g d) -> p g d", d=a_dim)
                    hm_v = hm_sb[:].rearrange("p (g d) -> p g d", d=out_dim)
                    w_v = w_sb[:].rearrange("p (g d) -> p g d", d=out_dim)
                    # tanh(z) for gn columns -> t_sb (ACT, psum read)
                    nc.scalar.activation(
                        out=t_v[:, g0 : g0 + gn, :],
                        in_=pzv[:, :gn, :a_dim],
                        func=mybir.ActivationFunctionType.Tanh,
                    )
                    # evict hm half of psum -> contiguous fp16 (ACT copy)
                    nc.scalar.activation(
                        out=hm_v[:, g0 : g0 + gn, :],
                        in_=pzv[:, :gn, a_dim:],
                        func=mybir.ActivationFunctionType.Copy,
                    )
                    tv = work.tile([P, SUBG * a_dim], f16, tag="tv")
                    nc.vector.tensor_tensor(
                        out=tv[:].rearrange("p (g d) -> p g d", d=a_dim)[
                            :, :gn, :
                        ],
                        in0=t_v[:, g0 : g0 + gn, :],
                        in1=vb_sb[:].unsqueeze(1).to_broadcast(
                            [P, gn, a_dim]
                        ),
                        op=mybir.AluOpType.mult,
                    )
                    nc.vector.tensor_reduce(
                        out=e_sb[:, g0 : g0 + gn],
                        in_=tv[:].rearrange("p (g d) -> p g d", d=a_dim)[
                            :, :gn, :
                        ],
                        axis=mybir.AxisListType.X,
                        op=mybir.AluOpType.add,
                    )
                    e2 = small.tile([P, SUBG], f32, tag="e2")
                    nc.vector.tensor_tensor(
                        out=e2[:, :gn],
                        in0=e_sb[:, g0 : g0 + gn],
                        in1=mask_sb[:, off + g0 : off + g0 + gn],
                        op=mybir.AluOpType.add,
                    )
                    nc.scalar.activation(
                        out=p_sb[:, g0 : g0 + gn],
                        in_=e2[:, :gn],
                        func=mybir.ActivationFunctionType.Exp,
                    )
                    # w[p, (g,d)] = p[p, g] * hm[p, (g,d)]
                    nc.vector.tensor_tensor(
                        out=w_v[:, g0 : g0 + gn, :],
                        in0=hm_v[:, g0 : g0 + gn, :],
                        in1=p_sb[:, g0 : g0 + gn]
                        .unsqueeze(2)
                        .to_broadcast([P, gn, out_dim]),
                        op=mybir.AluOpType.mult,
                    )
                    # accumulate later sub-batch slabs onto the first (one
                    # wide contiguous fp16 add each; 2x mode)
                    if sb_i > 0:
                        nc.vector.tensor_tensor(
                            out=w_sb[:, : gn * out_dim],
                            in0=w_sb[:, : gn * out_dim],
                            in1=w_sb[:, g0 * out_dim : (g0 + gn) * out_dim],
                            op=mybir.AluOpType.add,
                        )
                # fold-tree sum over the first slab, once per block:
                # contiguous fp16 halving adds stay in the DVE 2x mode
                gf = min(SUBG, db)
                while gf > 1:
                    if gf % 2 == 1:
                        nc.vector.tensor_tensor(
                            out=w_sb[:, :out_dim],
                            in0=w_sb[:, :out_dim],
                            in1=w_sb[:, (gf - 1) * out_dim : gf * out_dim],
                            op=mybir.AluOpType.add,
                        )
                        gf -= 1
                    half = gf // 2
                    nc.vector.tensor_tensor(
                        out=w_sb[:, : half * out_dim],
                        in0=w_sb[:, : half * out_dim],
                        in1=w_sb[:, half * out_dim : 2 * half * out_dim],
                        op=mybir.AluOpType.add,
                    )
                    gf = half
                denom = small.tile([P, 1], f32, tag="denom")
                nc.vector.tensor_reduce(
                    out=denom[:], in_=p_sb[:], axis=mybir.AxisListType.X,
                    op=mybir.AluOpType.add,
                )
                r_sb = small.tile([P, 1], f32, tag="r")
                nc.vector.reciprocal(out=r_sb[:], in_=denom[:])
                gi = b % ob_group
                if gi == 0:
                    out_t = outp.tile([P, ob_group * out_dim], f32, tag="ot")
                # out = tanh(numer * (1/denom)): the scale rides on ACT
                nc.scalar.activation(
                    out=out_t[:, gi * out_dim : (gi + 1) * out_dim],
                    in_=w_sb[:, :out_dim],
                    func=mybir.ActivationFunctionType.Tanh,
                    scale=r_sb[:],
                )
                if gi == ob_group - 1 or b == n_blocks - 1:
                    ng = gi + 1
                    b0 = b - gi
                    nc.sync.dma_start(
                        out=bass.AP(
                            tensor=out_d,
                            offset=b0 * P * out_dim,
                            ap=[[out_dim, P], [P * out_dim, ng], [1, out_dim]],
                        ),
                        in_=out_t[:].rearrange("p (g d) -> p g d", d=out_dim)[
                            :, :ng, :
                        ],
                    )
    nc.compile()
    return nc


_CACHE = {}


def _get_program(meta):
    key = (
        meta["own_pad"], meta["n_blocks"],
        tuple(int(x) for x in meta["d_blocks"]),
    )
    if key not in _CACHE:
        _CACHE[key] = _build_program(
            meta["n_blocks"], meta["d_blocks"], meta["col_of_block"],
            meta["own_pad"],
        )
    return _CACHE[key]


def run(h, edge_index, W_msg, Ws, Wd, v, trace=False, trace_kwargs=None):
    in_maps, meta = _preprocess(h, edge_index, W_msg, Ws, Wd, v, NCORES)
    nc = _get_program(meta)
    kwargs = {}
    if trace:
        kwargs = dict(trace=True, **(trace_kwargs or {}))
    res = run_bass_kernel_spmd(nc, in_maps, list(range(NCORES)), **kwargs)
    n, own = meta["n"], meta["own"]
    out_dim = res.results[0]["out"].shape[1]
    full = np.zeros((n, out_dim), dtype=np.float32)
    for c in range(NCORES):
        perm = meta["perms"][c]
        full[c * own + perm] = res.results[c]["out"][:own]
    return full, res


def kernel(h, edge_index, W_msg, Ws, Wd, v):
    out, _ = run(h, edge_index, W_msg, Ws, Wd, v)
    return out
